# revision 7
# baseline (speedup 1.0000x reference)
"""Trainium2 Bass kernel for nn_ContinuousActor (GNN message passing actor MLP).

Strategy (pure data parallel over 8 cores, batch dim sharded):
  - Host packs, per pair-couple c in {0,1,2}, an input tensor xt2[c] [128, B]
    holding the 53 features of pair 2c in rows 0-52 and of pair 2c+1 in rows
    64-116 (feature-major, with a ones row folding the phi1 bias+one-hots).
    The two pairs' phi1 matmuls (k=53) then run CONCURRENTLY on the PE via
    row-group tiling (base partitions 0 and 64), halving phi1's PE time.
  - Per 512-col batch tile: 3 couples x (2x phi1 -> relu -> 2x phi2 ->
    fused relu+accumulate) -> combine -> rho -> heads+clip.
  - phi2 psum per couple is a [128, 1024] 2-bank tile (pair-even cols 0-511,
    pair-odd 512-1023) so each drain is one wide op:
      m0 half: ACT relu+bias -> rm2, GPSIMD adds into split accumulator
      m1 half: DVE scalar_tensor_tensor (max(x,-b2) add acc2) fused
    The split accumulators (even-pair sums | odd-pair sums) are combined by
    GPSIMD before rho; the skipped +b2 of the m1 half is folded into the rho
    bias (asymmetric fold: br_eff = br + 6*b2_m1 @ wr_m1).
  - Software-pipelined one couple-block deep (phi2-m1 trails two blocks) so
    psum drains never stall the PE; initial DMAs issue in parallel from
    sync/scalar/vector queues so compute starts ~5us earlier.
  - Output stored bf16 [8, bc]; host transposes/upcasts.
"""

import numpy as np
import ml_dtypes
from collections import deque
from contextlib import ExitStack

import concourse.bass as bass
import concourse.mybir as mybir
import concourse.tile as tile
from concourse import bacc
from concourse.bass_utils import run_bass_kernel_spmd

F32 = mybir.dt.float32
BF16 = mybir.dt.bfloat16
RELU = mybir.ActivationFunctionType.Relu
NPBF = ml_dtypes.bfloat16

B_FULL = 65536
N_CORES = 8
BC = B_FULL // N_CORES  # 8192 batch rows per core
BT = 512                # batch tile (matmul free dim)
PERMS = [(0, 1), (0, 2), (1, 0), (1, 2), (2, 0), (2, 1)]
LOG_SIG_MIN, LOG_SIG_MAX = -20.0, 2.0

_CACHE = {}


def _pair_block_w(phi_w1, phi_b1, i, j):
    """[53, 256] weight block for pair (i,j): rows = [ag_i, g_i, obj_i,
    body, ones, ag_j, g_j, obj_j]."""
    f = np.float32
    rows = [phi_w1[0:3], phi_w1[6:9], phi_w1[25:40], phi_w1[12:22],
            (phi_b1 + phi_w1[22 + i] + phi_w1[40 + j]).reshape(1, -1),
            phi_w1[3:6], phi_w1[9:12], phi_w1[43:58]]
    return np.concatenate(rows, axis=0).astype(f)


def _pack_weights(phi_w1, phi_b1, phi_w2, phi_b2, rho_w1, rho_b1,
                  mean_w, mean_b, logstd_w, logstd_b):
    f = np.float32
    phi_w1, phi_b1 = np.asarray(phi_w1, f), np.asarray(phi_b1, f)
    phi_w2, phi_b2 = np.asarray(phi_w2, f), np.asarray(phi_b2, f)
    rho_w1, rho_b1 = np.asarray(rho_w1, f), np.asarray(rho_b1, f)
    # w1t: [128, 3*256]; couple c cols [c*256:(c+1)*256], even pair rows 0-52,
    # odd pair rows 64-116.
    w1t = np.zeros((128, 3 * 256), dtype=f)
    for c in range(3):
        i0, j0 = PERMS[2 * c]
        i1, j1 = PERMS[2 * c + 1]
        w1t[0:53, c * 256:(c + 1) * 256] = _pair_block_w(phi_w1, phi_b1, i0, j0)
        w1t[64:117, c * 256:(c + 1) * 256] = _pair_block_w(phi_w1, phi_b1, i1, j1)

    def pack_256(w):
        out = np.empty((128, 512), dtype=f)
        for k in range(2):
            for m in range(2):
                out[:, (2 * k + m) * 128:(2 * k + m + 1) * 128] = \
                    w[k * 128:(k + 1) * 128, m * 128:(m + 1) * 128]
        return out
    wh_full = np.concatenate([np.asarray(mean_w, f), np.asarray(logstd_w, f)], axis=1)
    wh = np.concatenate([wh_full[0:128, :], wh_full[128:256, :]], axis=1)  # [128, 16]
    wbig = np.concatenate([pack_256(phi_w2), pack_256(rho_w1), wh], axis=1)  # [128, 1040]
    bh = np.concatenate([np.asarray(mean_b, f), np.asarray(logstd_b, f)])
    small = np.concatenate([bh, np.ones(BT, f)]).reshape(1, 8 + BT)
    br_eff = rho_b1 + 6.0 * (phi_b2[128:256] @ rho_w1[128:256, :])
    big = np.float32(3.0e38)
    cpack = np.zeros((128, 6), dtype=f)
    cpack[:, 0] = -phi_b2[128:256]
    cpack[:, 1] = phi_b2[0:128]
    cpack[:, 2] = br_eff[0:128]
    cpack[:, 3] = br_eff[128:256]
    cpack[0:4, 4], cpack[4:8, 4] = big, LOG_SIG_MAX
    cpack[0:4, 5], cpack[4:8, 5] = -big, LOG_SIG_MIN
    return dict(w1t=w1t.astype(NPBF), wbig=wbig.astype(NPBF),
                small=small.astype(NPBF), cpack=cpack)


def _pack_xt2(obs, ag, g):
    """[3, 128, B] bf16 couple input tensors."""
    B = obs.shape[0]
    f = np.float32
    OBJ_IDS = np.array([[0, 1, 2], [3, 4, 5], [6, 7, 8]])
    ones = np.ones((1, B), f)
    body = obs.T[0:10]

    def pair_rows(i, j):
        return np.concatenate([
            ag.T[OBJ_IDS[i]], g.T[OBJ_IDS[i]], obs.T[10 + 15 * i:25 + 15 * i],
            body, ones,
            ag.T[OBJ_IDS[j]], g.T[OBJ_IDS[j]], obs.T[10 + 15 * j:25 + 15 * j],
        ], axis=0)  # [53, B]

    xt2 = np.zeros((3, 128, B), dtype=NPBF)
    for c in range(3):
        i0, j0 = PERMS[2 * c]
        i1, j1 = PERMS[2 * c + 1]
        xt2[c, 0:53] = pair_rows(i0, j0).astype(NPBF)
        xt2[c, 64:117] = pair_rows(i1, j1).astype(NPBF)
    return xt2


def _build_bass(bc, bt):
    nt = bc // bt
    nb = nt * 3  # couple-blocks
    nc = bacc.Bacc(trn_type="TRN2")

    xt2_d = [nc.dram_tensor(f"xt2_{c}", [128, bc], BF16, kind="ExternalInput")
             for c in range(3)]
    w1t_d = nc.dram_tensor("w1t", [128, 3 * 256], BF16, kind="ExternalInput")
    wbig_d = nc.dram_tensor("wbig", [128, 1040], BF16, kind="ExternalInput")
    small_d = nc.dram_tensor("small", [1, 8 + bt], BF16, kind="ExternalInput")
    cpack_d = nc.dram_tensor("cpack", [128, 6], F32, kind="ExternalInput")
    y_d = nc.dram_tensor("y", [8, bc], BF16, kind="ExternalOutput")

    AMIN, AMAX, AADD = mybir.AluOpType.min, mybir.AluOpType.max, mybir.AluOpType.add

    with ExitStack() as ctx:
        tc = ctx.enter_context(tile.TileContext(nc))
        consts = ctx.enter_context(tc.tile_pool(name="consts", bufs=1))
        sbp = ctx.enter_context(tc.tile_pool(name="sbp", bufs=2))
        psp = ctx.enter_context(tc.tile_pool(name="psp", bufs=2, space="PSUM"))

        # xt2 group loads: one [128, 2*bt] DMA per couple per 2 tiles
        xt2_tiles = {}

        def load_xt2(grp, eng):
            for c in range(3):
                x = sbp.tile([128, 2 * bt], BF16, tag=f"xt2_{c}", name="x2",
                             bufs=3)
                eng.dma_start(out=x, in_=xt2_d[c][:, grp * 2 * bt:(grp + 1) * 2 * bt])
                xt2_tiles[(grp, c)] = x

        # initial loads spread across queues so compute starts ASAP
        load_xt2(0, nc.sync)
        w1sb = consts.tile([128, 3 * 256], BF16)
        nc.sync.dma_start(out=w1sb, in_=w1t_d[:, :])
        wbsb = consts.tile([128, 1040], BF16)
        nc.scalar.dma_start(out=wbsb, in_=wbig_d[:, :])
        smsb = consts.tile([1, 8 + bt], BF16)
        nc.scalar.dma_start(out=smsb, in_=small_d[:, :])
        cpsb = consts.tile([128, 6], F32)
        nc.scalar.dma_start(out=cpsb, in_=cpack_d[:, :])
        load_xt2(1, nc.scalar)

        w2sb = wbsb[:, 0:512]
        wrsb = wbsb[:, 512:1024]
        whsb = wbsb[:, 1024:1040]
        bhsb = smsb[:, 0:8]
        ones_sb = smsb[:, 8:8 + bt]

        ph1s, h1s = {}, {}
        acc2m0, acc2m1 = {}, {}   # per tile: [128, 2*bt] split accumulators
        accm = {}                 # per tile: combined [128, 2*bt] (m0|m1)
        fin_q = deque()

        def xslice(t, c):
            x = xt2_tiles[(t // 2, c)]
            o = (t % 2) * bt
            return x[:, o:o + bt]

        def stage1(cb):
            """phi1 for couple cb: 4 matmuls (2 row-tiled concurrent pairs)."""
            t, cp = divmod(cb, 3)
            if cp == 0:
                if t % 2 == 0 and t + 4 < nt:
                    load_xt2(t // 2 + 2, nc.sync)
                acc2m0[t] = sbp.tile([128, 2 * bt], BF16, tag="a2m0", name="a0")
                acc2m1[t] = sbp.tile([128, 2 * bt], BF16, tag="a2m1", name="a1")
            xe = xslice(t, cp)
            phe = psp.tile([128, 2 * bt], F32, tag="ph1", name="phe")
            pho = psp.tile([128, 2 * bt], F32, tag="ph1", name="pho")
            for m in range(2):
                wcol = cp * 256 + m * 128
                nc.tensor.matmul(
                    phe[:, m * bt:(m + 1) * bt],
                    w1sb[0:53, wcol:wcol + 128], xe[0:53, :],
                    start=True, stop=True,
                )
                nc.tensor.matmul(
                    pho[:, m * bt:(m + 1) * bt],
                    w1sb[64:117, wcol:wcol + 128], xe[64:117, :],
                    start=True, stop=True,
                )
            h1e = sbp.tile([128, 2 * bt], BF16, tag="h1", name="h1e", bufs=4)
            h1o = sbp.tile([128, 2 * bt], BF16, tag="h1", name="h1o", bufs=4)
            nc.scalar.activation(h1e, phe, RELU)
            nc.vector.tensor_scalar_max(h1o, pho, 0.0)
            ph1s[cb], h1s[cb] = (phe, pho), (h1e, h1o)

        def stage2(cb):
            """phi2-m0 for couple cb: [128, 2*bt] psum, one ACT drain, GP add."""
            t, cp = divmod(cb, 3)
            h1e, h1o = h1s[cb]
            pha = psp.tile([128, 2 * bt], F32, tag="pha", name="pha", bufs=1)
            for half, h1 in ((0, h1e), (1, h1o)):
                for k in range(2):
                    nc.tensor.matmul(
                        pha[:, half * bt:(half + 1) * bt],
                        w2sb[:, (2 * k) * 128:(2 * k + 1) * 128],
                        h1[:, k * bt:(k + 1) * bt],
                        start=(k == 0), stop=(k == 1),
                    )
            if cp == 0:
                nc.scalar.activation(acc2m0[t], pha, RELU, bias=cpsb[:, 1:2])
            else:
                rm2 = sbp.tile([128, 2 * bt], BF16, tag="rm2", name="rm2", bufs=2)
                nc.scalar.activation(rm2, pha, RELU, bias=cpsb[:, 1:2])
                nc.gpsimd.tensor_add(acc2m0[t], acc2m0[t], rm2)
            if cp == 2:  # last couple: combine split accumulators (m0)
                am = accm[t] = sbp.tile([128, 2 * bt], BF16, tag="accm", name="am")
                nc.gpsimd.tensor_add(am[:, 0:bt], acc2m0[t][:, 0:bt],
                                     acc2m0[t][:, bt:2 * bt])

        def stage3(cb):
            """phi2-m1 for couple cb: [128, 2*bt] psum, one fused DVE drain."""
            t, cp = divmod(cb, 3)
            h1e, h1o = h1s[cb]
            phb = psp.tile([128, 2 * bt], F32, tag="phb", name="phb", bufs=1)
            for half, h1 in ((0, h1e), (1, h1o)):
                for k in range(2):
                    nc.tensor.matmul(
                        phb[:, half * bt:(half + 1) * bt],
                        w2sb[:, (2 * k + 1) * 128:(2 * k + 2) * 128],
                        h1[:, k * bt:(k + 1) * bt],
                        start=(k == 0), stop=(k == 1),
                    )
            if cp == 0:
                nc.vector.tensor_scalar(
                    acc2m1[t], phb, cpsb[:, 0:1], 0.0, op0=AMAX, op1=AADD)
            else:
                nc.vector.scalar_tensor_tensor(
                    acc2m1[t], phb, cpsb[:, 0:1], acc2m1[t],
                    op0=AMAX, op1=AADD)
            if cp == 2:  # combine split accumulators (m1)
                am = accm[t]
                nc.gpsimd.tensor_add(am[:, bt:2 * bt], acc2m1[t][:, 0:bt],
                                     acc2m1[t][:, bt:2 * bt])
            del ph1s[cb], h1s[cb]

        def finisher(t):
            am = accm[t]
            st = {}

            def stage_ab():  # rho m0+m1 into one [128, 2*bt] psum tile
                pr2 = psp.tile([128, 2 * bt], F32, tag="pha", name="pr2", bufs=1)
                for m in range(2):
                    for k in range(2):
                        nc.tensor.matmul(
                            pr2[:, m * bt:(m + 1) * bt],
                            wrsb[:, (2 * k + m) * 128:(2 * k + m + 1) * 128],
                            am[:, k * bt:(k + 1) * bt],
                            start=(k == 0), stop=(k == 1),
                        )
                xs = sbp.tile([128, 2 * bt], BF16, tag="xs", name="xs")
                nc.scalar.activation(xs[:, 0:bt], pr2[:, 0:bt], RELU,
                                     bias=cpsb[:, 2:3])
                nc.scalar.activation(xs[:, bt:2 * bt], pr2[:, bt:2 * bt], RELU,
                                     bias=cpsb[:, 3:4])
                st["xs"] = xs

            def stage_c():  # heads + clip + store
                xs = st["xs"]
                py = psp.tile([8, bt], F32, tag="phb", name="py", bufs=1)
                for k in range(2):
                    nc.tensor.matmul(py, whsb[:, k * 8:(k + 1) * 8],
                                     xs[:, k * bt:(k + 1) * bt],
                                     start=(k == 0), stop=False)
                nc.tensor.matmul(py, bhsb, ones_sb, start=False, stop=True)
                ysb = sbp.tile([8, bt], BF16, tag="ysb", name="ysb")
                nc.vector.tensor_scalar(
                    ysb, py, cpsb[0:8, 4:5], cpsb[0:8, 5:6],
                    op0=AMIN, op1=AMAX)
                nc.sync.dma_start(out=y_d[:, t * bt:(t + 1) * bt], in_=ysb)

            return [stage_ab, stage_c]

        # --- master emission loop (couple-blocks, 1-block software skew).
        # Finisher pops BEFORE stage2/3 so borrowed psum slots (pha/phb tags)
        # are one block old when the finisher matmuls claim them. ---
        for cb in range(nb + 2):
            if cb < nb:
                stage1(cb)
            if fin_q:
                fin_q.popleft()()
            if 0 <= cb - 1 < nb:
                stage2(cb - 1)
            if 0 <= cb - 2 < nb:
                stage3(cb - 2)
            if cb - 2 >= 0 and (cb - 2) % 3 == 2:
                fin_q.extend(finisher((cb - 2) // 3))
        while fin_q:
            fin_q.popleft()()

    return nc


def _get_nc(bc, bt):
    key = (bc, bt)
    if key not in _CACHE:
        nc = _build_bass(bc, bt)
        nc.finalize()
        _CACHE[key] = nc
    return _CACHE[key]


def kernel(obs, ag, g, phi_w1, phi_b1, phi_w2, phi_b2,
           rho_w1, rho_b1, mean_w, mean_b, logstd_w, logstd_b):
    obs = np.asarray(obs, np.float32)
    ag = np.asarray(ag, np.float32)
    g = np.asarray(g, np.float32)
    B = obs.shape[0]
    assert B == B_FULL, f"kernel hardcoded for B={B_FULL}, got {B}"

    packed = _pack_weights(phi_w1, phi_b1, phi_w2, phi_b2, rho_w1, rho_b1,
                           mean_w, mean_b, logstd_w, logstd_b)
    xt2 = _pack_xt2(obs, ag, g)

    nc = _get_nc(BC, BT)
    in_maps = []
    for c in range(N_CORES):
        m = dict(packed)
        for cc in range(3):
            m[f"xt2_{cc}"] = np.ascontiguousarray(xt2[cc][:, c * BC:(c + 1) * BC])
        in_maps.append(m)

    import os
    trace = bool(os.environ.get("KERNEL_TRACE"))
    res = run_bass_kernel_spmd(nc, in_maps, core_ids=list(range(N_CORES)),
                               trace=trace)
    global _last_results
    _last_results = res

    y = np.concatenate(
        [np.asarray(res.results[c]["y"]) for c in range(N_CORES)], axis=1)
    out = np.ascontiguousarray(y.T.astype(np.float32))  # [B, 8]
    mean = out[:, 0:4].copy()
    logstd = out[:, 4:8].copy()
    return mean, logstd


_last_results = None


# revision 10
# speedup vs baseline: 1.5419x; 1.5419x over previous
"""Trainium2 Bass kernel for nn_ContinuousActor (GNN message passing actor MLP).

Strategy (pure data parallel over 8 cores, batch dim sharded):
  - Host repacks inputs feature-major: XT[74, B] = [obs.T; ag.T; g.T; ones].
    The ones row folds the per-pair phi1 bias (incl. one-hot rows) into the
    matmul, so the phi1 psum drain is a pure relu.
  - Per-pair effective phi1 weights W1e[p] [74, 256] built host-side.
  - Device pipeline per 512-col batch tile (feature-major [feat, batch]):
    6x (phi1 -> relu -> phi2 -> fused relu+accumulate) -> rho -> heads+clip.
    Software-pipelined one pair deep: pair p+1's phi1 matmuls are emitted
    before pair p's phi2-m0, and phi2-m1 trails one more block, giving the
    psum drains a ~1.7us window so the PE never waits on a drain round trip.
  - Bias handling: phi2-m0 half via ACT relu+bias; phi2-m1 half via DVE
    scalar_tensor_tensor (max(x,-b2) add acc) with the missing 6*b2_m1
    correction folded into the rho bias (asymmetric fold).
  - Pooling accumulates in bf16: m0 via GPSIMD adds, m1 fused in the STT op.
  - Output stored bf16 [8, bc]; host transposes/upcasts.
"""

import numpy as np
import ml_dtypes
from collections import deque
from contextlib import ExitStack

import concourse.bass as bass
import concourse.mybir as mybir
import concourse.tile as tile
from concourse import bacc
from concourse.bass_utils import run_bass_kernel_spmd

F32 = mybir.dt.float32
BF16 = mybir.dt.bfloat16
RELU = mybir.ActivationFunctionType.Relu
NPBF = ml_dtypes.bfloat16

B_FULL = 65536
N_CORES = 8
BC = B_FULL // N_CORES  # 8192 batch rows per core
BT = 512                # batch tile (matmul free dim)
KX = 74                 # 55 obs + 9 ag + 9 g + 1 ones
PERMS = [(0, 1), (0, 2), (1, 0), (1, 2), (2, 0), (2, 1)]
LOG_SIG_MIN, LOG_SIG_MAX = -20.0, 2.0

_CACHE = {}


def _pack_weights(phi_w1, phi_b1, phi_w2, phi_b2, rho_w1, rho_b1,
                  mean_w, mean_b, logstd_w, logstd_b):
    """Host-side weight repacking into device layouts."""
    f = np.float32
    phi_w1, phi_b1 = np.asarray(phi_w1, f), np.asarray(phi_b1, f)
    phi_w2, phi_b2 = np.asarray(phi_w2, f), np.asarray(phi_b2, f)
    rho_w1, rho_b1 = np.asarray(rho_w1, f), np.asarray(rho_b1, f)
    # w1e: per-pair effective weights [74, 6*256]; ones-row (73) carries bias.
    w1 = np.zeros((KX, 6 * 256), dtype=f)
    for p, (i, j) in enumerate(PERMS):
        Wp = w1[:, p * 256:(p + 1) * 256]
        Wp[0:10] = phi_w1[12:22]                      # obs body
        Wp[10 + 15 * i:25 + 15 * i] = phi_w1[25:40]   # obj i features
        Wp[10 + 15 * j:25 + 15 * j] = phi_w1[43:58]   # obj j features
        Wp[55 + 3 * i:58 + 3 * i] = phi_w1[0:3]       # ag_i
        Wp[55 + 3 * j:58 + 3 * j] = phi_w1[3:6]       # ag_j
        Wp[64 + 3 * i:67 + 3 * i] = phi_w1[6:9]       # g_i
        Wp[64 + 3 * j:67 + 3 * j] = phi_w1[9:12]      # g_j
        Wp[73] = phi_b1 + phi_w1[22 + i] + phi_w1[40 + j]  # bias + one-hots

    # w2/wr: [128, 512] with col block (2k+m) = W[k*128:(k+1)*128, m*128:(m+1)*128]
    def pack_256(w):
        out = np.empty((128, 512), dtype=f)
        for k in range(2):
            for m in range(2):
                out[:, (2 * k + m) * 128:(2 * k + m + 1) * 128] = \
                    w[k * 128:(k + 1) * 128, m * 128:(m + 1) * 128]
        return out
    # wbig = [w2p | wrp | wh] : [128, 512+512+16]
    wh_full = np.concatenate([np.asarray(mean_w, f), np.asarray(logstd_w, f)], axis=1)  # [256, 8]
    wh = np.concatenate([wh_full[0:128, :], wh_full[128:256, :]], axis=1)  # [128, 16]
    wbig = np.concatenate([pack_256(phi_w2), pack_256(rho_w1), wh], axis=1)  # [128, 1040]
    # small: row vector [1, 520] = [bh(8) | ones(512)]
    bh = np.concatenate([np.asarray(mean_b, f), np.asarray(logstd_b, f)])  # [8]
    small = np.concatenate([bh, np.ones(BT, f)]).reshape(1, 8 + BT)
    # cpack [128, 6] f32:
    #  c0 = -b2_m1 (STT max threshold), c1 = b2_m0 (ACT bias),
    #  c2/c3 = br_eff m0/m1, c4/c5 = head clip hi/lo (rows 0-7)
    br_eff = rho_b1 + 6.0 * (phi_b2[128:256] @ rho_w1[128:256, :])  # [256]
    big = np.float32(3.0e38)
    cpack = np.zeros((128, 6), dtype=f)
    cpack[:, 0] = -phi_b2[128:256]
    cpack[:, 1] = phi_b2[0:128]
    cpack[:, 2] = br_eff[0:128]
    cpack[:, 3] = br_eff[128:256]
    cpack[0:4, 4], cpack[4:8, 4] = big, LOG_SIG_MAX   # hi (min)
    cpack[0:4, 5], cpack[4:8, 5] = -big, LOG_SIG_MIN  # lo (max)
    return dict(w1=w1.astype(NPBF), wbig=wbig.astype(NPBF),
                small=small.astype(NPBF), cpack=cpack)


def _pack_xt(obs, ag, g):
    B = obs.shape[0]
    xt = np.empty((KX, B), dtype=NPBF)
    xt[0:55] = obs.T.astype(NPBF)
    xt[55:64] = ag.T.astype(NPBF)
    xt[64:73] = g.T.astype(NPBF)
    xt[73] = np.asarray(1.0, NPBF)
    return xt


def _build_bass(bc, bt):
    nt = bc // bt
    nq = nt * 6  # global pair count
    nc = bacc.Bacc(trn_type="TRN2")

    xt_d = nc.dram_tensor("xt", [KX, bc], BF16, kind="ExternalInput")
    w1_d = nc.dram_tensor("w1", [KX, 6 * 256], BF16, kind="ExternalInput")
    wbig_d = nc.dram_tensor("wbig", [128, 1040], BF16, kind="ExternalInput")
    small_d = nc.dram_tensor("small", [1, 8 + bt], BF16, kind="ExternalInput")
    cpack_d = nc.dram_tensor("cpack", [128, 6], F32, kind="ExternalInput")
    y_d = nc.dram_tensor("y", [8, bc], BF16, kind="ExternalOutput")

    AMIN, AMAX, AADD = mybir.AluOpType.min, mybir.AluOpType.max, mybir.AluOpType.add

    with ExitStack() as ctx:
        tc = ctx.enter_context(tile.TileContext(nc))
        consts = ctx.enter_context(tc.tile_pool(name="consts", bufs=1))
        sbp = ctx.enter_context(tc.tile_pool(name="sbp", bufs=2))
        psp = ctx.enter_context(tc.tile_pool(name="psp", bufs=2, space="PSUM"))

        # --- const loads: xts(0) first so compute starts ASAP -------------
        xts_tiles = {}

        def load_xts(t):
            xts = sbp.tile([KX, bt], BF16, tag="xts", name="xts", bufs=3)
            nc.sync.dma_start(out=xts, in_=xt_d[:, t * bt:(t + 1) * bt])
            xts_tiles[t] = xts

        load_xts(0)
        w1sb = consts.tile([KX, 6 * 256], BF16)
        nc.sync.dma_start(out=w1sb, in_=w1_d[:, :])
        wbsb = consts.tile([128, 1040], BF16)
        nc.scalar.dma_start(out=wbsb, in_=wbig_d[:, :])
        smsb = consts.tile([1, 8 + bt], BF16)
        nc.scalar.dma_start(out=smsb, in_=small_d[:, :])
        cpsb = consts.tile([128, 6], F32)
        nc.scalar.dma_start(out=cpsb, in_=cpack_d[:, :])
        load_xts(1)
        load_xts(2)

        w2sb = wbsb[:, 0:512]
        wrsb = wbsb[:, 512:1024]
        whsb = wbsb[:, 1024:1040]
        bhsb = smsb[:, 0:8]
        ones_sb = smsb[:, 8:8 + bt]

        # --- per-pair state ----------------------------------------------
        ph1s, h1s, accs = {}, {}, {}
        fin_q = deque()

        def stage1(q):
            """phi1 matmuls for global pair q + psum drain (pure relu)."""
            t, p = divmod(q, 6)
            if p == 0:
                if t + 3 < nt:
                    load_xts(t + 3)
                accs[t] = sbp.tile([128, 2 * bt], BF16, tag="acc", name="acc")
            xts = xts_tiles[t]
            ph1 = psp.tile([128, 2 * bt], F32, tag="ph1", name="ph1")
            for m in range(2):
                nc.tensor.matmul(
                    ph1[:, m * bt:(m + 1) * bt],
                    w1sb[:, p * 256 + m * 128:p * 256 + (m + 1) * 128],
                    xts, start=True, stop=True,
                )
            h1 = sbp.tile([128, 2 * bt], BF16, tag="h1", name="h1", bufs=3)
            if p in (1, 3, 5):
                nc.vector.tensor_scalar_max(h1, ph1, 0.0)
            else:
                nc.scalar.activation(h1, ph1, RELU)
            ph1s[q], h1s[q] = ph1, h1

        def stage2(q):
            """phi2 m0 matmuls for pair q + ACT relu+bias consumer."""
            t, p = divmod(q, 6)
            h1, acc = h1s[q], accs[t]
            pha = psp.tile([128, bt], F32, tag="pha", name="pha")
            for k in range(2):
                nc.tensor.matmul(
                    pha, w2sb[:, (2 * k) * 128:(2 * k + 1) * 128],
                    h1[:, k * bt:(k + 1) * bt], start=(k == 0), stop=(k == 1),
                )
            if p == 0:
                nc.scalar.activation(acc[:, 0:bt], pha, RELU, bias=cpsb[:, 1:2])
            else:
                rm0 = sbp.tile([128, bt], BF16, tag="rm0", name="rm0", bufs=3)
                nc.scalar.activation(rm0, pha, RELU, bias=cpsb[:, 1:2])
                nc.gpsimd.tensor_add(acc[:, 0:bt], acc[:, 0:bt], rm0)

        def stage3(q):
            """phi2 m1 matmuls for pair q + DVE fused relu/accumulate."""
            t, p = divmod(q, 6)
            h1, acc = h1s[q], accs[t]
            phb = psp.tile([128, bt], F32, tag="phb", name="phb")
            for k in range(2):
                nc.tensor.matmul(
                    phb, w2sb[:, (2 * k + 1) * 128:(2 * k + 2) * 128],
                    h1[:, k * bt:(k + 1) * bt], start=(k == 0), stop=(k == 1),
                )
            if p == 0:
                nc.vector.tensor_scalar(
                    acc[:, bt:2 * bt], phb, cpsb[:, 0:1], 0.0,
                    op0=AMAX, op1=AADD,
                )
            else:
                nc.vector.scalar_tensor_tensor(
                    acc[:, bt:2 * bt], phb, cpsb[:, 0:1], acc[:, bt:2 * bt],
                    op0=AMAX, op1=AADD,
                )
            del ph1s[q], h1s[q]

        def finisher(t):
            """rho + heads + clip + store for tile t, as 3 weavable stages."""
            acc = accs[t]
            st = {}

            def stage_a():  # rho m0
                pr0 = psp.tile([128, bt], F32, tag="phb", name="pr0", bufs=2)
                for k in range(2):
                    nc.tensor.matmul(
                        pr0, wrsb[:, (2 * k) * 128:(2 * k + 1) * 128],
                        acc[:, k * bt:(k + 1) * bt],
                        start=(k == 0), stop=(k == 1),
                    )
                xs = sbp.tile([128, 2 * bt], BF16, tag="xs", name="xs")
                nc.scalar.activation(xs[:, 0:bt], pr0, RELU, bias=cpsb[:, 2:3])
                st["xs"] = xs

            def stage_b():  # rho m1
                pr1 = psp.tile([128, bt], F32, tag="phb", name="pr1", bufs=2)
                for k in range(2):
                    nc.tensor.matmul(
                        pr1, wrsb[:, (2 * k + 1) * 128:(2 * k + 2) * 128],
                        acc[:, k * bt:(k + 1) * bt],
                        start=(k == 0), stop=(k == 1),
                    )
                nc.scalar.activation(st["xs"][:, bt:2 * bt], pr1, RELU,
                                     bias=cpsb[:, 3:4])

            def stage_c():  # heads + clip + store
                xs = st["xs"]
                py = psp.tile([8, bt], F32, tag="pha", name="py", bufs=2)
                for k in range(2):
                    nc.tensor.matmul(py, whsb[:, k * 8:(k + 1) * 8],
                                     xs[:, k * bt:(k + 1) * bt],
                                     start=(k == 0), stop=False)
                nc.tensor.matmul(py, bhsb, ones_sb, start=False, stop=True)
                ysb = sbp.tile([8, bt], BF16, tag="ysb", name="ysb")
                nc.vector.tensor_scalar(
                    ysb, py, cpsb[0:8, 4:5], cpsb[0:8, 5:6],
                    op0=AMIN, op1=AMAX,
                )
                nc.sync.dma_start(out=y_d[:, t * bt:(t + 1) * bt], in_=ysb)

            return [stage_a, stage_b, stage_c]

        # --- master emission loop (1-pair software skew) ------------------
        for q in range(nq + 2):
            t, p = divmod(q, 6)
            if q < nq:
                stage1(q)
            if 0 <= q - 1 < nq:
                stage2(q - 1)
            if 0 <= q - 2 < nq:
                stage3(q - 2)
            if p in (1, 3, 5) and fin_q:
                fin_q.popleft()()
            if q - 2 >= 0 and (q - 2) % 6 == 5:
                fin_q.extend(finisher((q - 2) // 6))
        while fin_q:
            fin_q.popleft()()

    return nc


def _get_nc(bc, bt):
    key = (bc, bt)
    if key not in _CACHE:
        nc = _build_bass(bc, bt)
        nc.finalize()
        _CACHE[key] = nc
    return _CACHE[key]


def kernel(obs, ag, g, phi_w1, phi_b1, phi_w2, phi_b2,
           rho_w1, rho_b1, mean_w, mean_b, logstd_w, logstd_b):
    obs = np.asarray(obs, np.float32)
    ag = np.asarray(ag, np.float32)
    g = np.asarray(g, np.float32)
    B = obs.shape[0]
    assert B == B_FULL, f"kernel hardcoded for B={B_FULL}, got {B}"

    packed = _pack_weights(phi_w1, phi_b1, phi_w2, phi_b2, rho_w1, rho_b1,
                           mean_w, mean_b, logstd_w, logstd_b)
    xt = _pack_xt(obs, ag, g)

    nc = _get_nc(BC, BT)
    in_maps = []
    for c in range(N_CORES):
        m = dict(packed)
        m["xt"] = np.ascontiguousarray(xt[:, c * BC:(c + 1) * BC])
        in_maps.append(m)

    import os
    trace = bool(os.environ.get("KERNEL_TRACE"))
    res = run_bass_kernel_spmd(nc, in_maps, core_ids=list(range(N_CORES)),
                               trace=trace)
    global _last_results
    _last_results = res

    y = np.concatenate(
        [np.asarray(res.results[c]["y"]) for c in range(N_CORES)], axis=1)
    out = np.ascontiguousarray(y.T.astype(np.float32))  # [B, 8]
    mean = out[:, 0:4].copy()
    logstd = out[:, 4:8].copy()
    return mean, logstd


_last_results = None


# revision 13
# speedup vs baseline: 1.5706x; 1.0186x over previous
"""Trainium2 Bass kernel for nn_ContinuousActor (GNN message passing actor MLP).

Strategy (pure data parallel over 8 cores, batch dim sharded):
  - Host repacks inputs feature-major: XT[74, B] = [obs.T; ag.T; g.T; ones].
    The ones row folds the per-pair phi1 bias (incl. one-hot rows) into the
    matmul, so the phi1 psum drain is a pure relu.
  - Per-pair effective phi1 weights W1e[p] [74, 256] built host-side.
  - Device pipeline per 512-col batch tile (feature-major [feat, batch]):
    6x (phi1 -> relu -> phi2 -> fused relu+accumulate) -> rho -> heads+clip.
    Software-pipelined one pair deep: pair p+1's phi1 matmuls are emitted
    before pair p's phi2-m0, and phi2-m1 trails one more block, giving the
    psum drains a ~1.7us window so the PE never waits on a drain round trip.
  - Bias handling: phi2-m0 half via ACT relu+bias; phi2-m1 half via DVE
    scalar_tensor_tensor (max(x,-b2) add acc) with the missing 6*b2_m1
    correction folded into the rho bias (asymmetric fold).
  - Pooling accumulates in bf16: m0 via GPSIMD adds, m1 fused in the STT op.
  - Output stored bf16 [8, bc]; host transposes/upcasts.
"""

import numpy as np
import ml_dtypes
from collections import deque
from contextlib import ExitStack

import concourse.bass as bass
import concourse.mybir as mybir
import concourse.tile as tile
from concourse import bacc
from concourse.bass_utils import run_bass_kernel_spmd

F32 = mybir.dt.float32
BF16 = mybir.dt.bfloat16
RELU = mybir.ActivationFunctionType.Relu
NPBF = ml_dtypes.bfloat16

B_FULL = 65536
N_CORES = 8
BC = B_FULL // N_CORES  # 8192 batch rows per core
BT = 512                # batch tile (matmul free dim)
KX = 74                 # 55 obs + 9 ag + 9 g + 1 ones
PERMS = [(0, 1), (0, 2), (1, 0), (1, 2), (2, 0), (2, 1)]
LOG_SIG_MIN, LOG_SIG_MAX = -20.0, 2.0

_CACHE = {}


def _pack_weights(phi_w1, phi_b1, phi_w2, phi_b2, rho_w1, rho_b1,
                  mean_w, mean_b, logstd_w, logstd_b):
    """Host-side weight repacking into device layouts."""
    f = np.float32
    phi_w1, phi_b1 = np.asarray(phi_w1, f), np.asarray(phi_b1, f)
    phi_w2, phi_b2 = np.asarray(phi_w2, f), np.asarray(phi_b2, f)
    rho_w1, rho_b1 = np.asarray(rho_w1, f), np.asarray(rho_b1, f)
    # w1e: per-pair effective weights [74, 6*256]; ones-row (73) carries bias.
    w1 = np.zeros((KX, 6 * 256), dtype=f)
    for p, (i, j) in enumerate(PERMS):
        Wp = w1[:, p * 256:(p + 1) * 256]
        Wp[0:10] = phi_w1[12:22]                      # obs body
        Wp[10 + 15 * i:25 + 15 * i] = phi_w1[25:40]   # obj i features
        Wp[10 + 15 * j:25 + 15 * j] = phi_w1[43:58]   # obj j features
        Wp[55 + 3 * i:58 + 3 * i] = phi_w1[0:3]       # ag_i
        Wp[55 + 3 * j:58 + 3 * j] = phi_w1[3:6]       # ag_j
        Wp[64 + 3 * i:67 + 3 * i] = phi_w1[6:9]       # g_i
        Wp[64 + 3 * j:67 + 3 * j] = phi_w1[9:12]      # g_j
        Wp[73] = phi_b1 + phi_w1[22 + i] + phi_w1[40 + j]  # bias + one-hots

    # w2/wr: [128, 512] with col block (2k+m) = W[k*128:(k+1)*128, m*128:(m+1)*128]
    def pack_256(w):
        out = np.empty((128, 512), dtype=f)
        for k in range(2):
            for m in range(2):
                out[:, (2 * k + m) * 128:(2 * k + m + 1) * 128] = \
                    w[k * 128:(k + 1) * 128, m * 128:(m + 1) * 128]
        return out
    # wbig = [w2p | wrp | wh] : [128, 512+512+16]
    wh_full = np.concatenate([np.asarray(mean_w, f), np.asarray(logstd_w, f)], axis=1)  # [256, 8]
    wh = np.concatenate([wh_full[0:128, :], wh_full[128:256, :]], axis=1)  # [128, 16]
    wbig = np.concatenate([pack_256(phi_w2), pack_256(rho_w1), wh], axis=1)  # [128, 1040]
    # small: row vector [1, 520] = [bh(8) | ones(512)]
    bh = np.concatenate([np.asarray(mean_b, f), np.asarray(logstd_b, f)])  # [8]
    small = np.concatenate([bh, np.ones(BT, f)]).reshape(1, 8 + BT)
    # cpack [128, 6] f32:
    #  c0 = -b2_m1 (STT max threshold), c1 = b2_m0 (ACT bias),
    #  c2/c3 = br_eff m0/m1, c4/c5 = head clip hi/lo (rows 0-7)
    br_eff = rho_b1 + 6.0 * (phi_b2[128:256] @ rho_w1[128:256, :])  # [256]
    big = np.float32(3.0e38)
    cpack = np.zeros((128, 6), dtype=f)
    cpack[:, 0] = -phi_b2[128:256]
    cpack[:, 1] = phi_b2[0:128]
    cpack[:, 2] = br_eff[0:128]
    cpack[:, 3] = br_eff[128:256]
    cpack[0:4, 4], cpack[4:8, 4] = big, LOG_SIG_MAX   # hi (min)
    cpack[0:4, 5], cpack[4:8, 5] = -big, LOG_SIG_MIN  # lo (max)
    return dict(w1=w1.astype(NPBF), wbig=wbig.astype(NPBF),
                small=small.astype(NPBF), cpack=cpack)


def _pack_xt(obs, ag, g):
    B = obs.shape[0]
    xt = np.empty((KX, B), dtype=NPBF)
    xt[0:55] = obs.T.astype(NPBF)
    xt[55:64] = ag.T.astype(NPBF)
    xt[64:73] = g.T.astype(NPBF)
    xt[73] = np.asarray(1.0, NPBF)
    return xt


def _build_bass(bc, bt):
    nt = bc // bt
    nq = nt * 6  # global pair count
    nc = bacc.Bacc(trn_type="TRN2")

    xt_d = nc.dram_tensor("xt", [KX, bc], BF16, kind="ExternalInput")
    w1_d = nc.dram_tensor("w1", [KX, 6 * 256], BF16, kind="ExternalInput")
    wbig_d = nc.dram_tensor("wbig", [128, 1040], BF16, kind="ExternalInput")
    small_d = nc.dram_tensor("small", [1, 8 + bt], BF16, kind="ExternalInput")
    cpack_d = nc.dram_tensor("cpack", [128, 6], F32, kind="ExternalInput")
    y_d = nc.dram_tensor("y", [8, bc], BF16, kind="ExternalOutput")

    AMIN, AMAX, AADD = mybir.AluOpType.min, mybir.AluOpType.max, mybir.AluOpType.add

    with ExitStack() as ctx:
        tc = ctx.enter_context(tile.TileContext(nc))
        consts = ctx.enter_context(tc.tile_pool(name="consts", bufs=1))
        sbp = ctx.enter_context(tc.tile_pool(name="sbp", bufs=2))
        psp = ctx.enter_context(tc.tile_pool(name="psp", bufs=2, space="PSUM"))

        # --- const loads: xts(0) first so compute starts ASAP -------------
        xts_tiles = {}

        def load_xts(t):
            xts = sbp.tile([KX, bt], BF16, tag="xts", name="xts", bufs=3)
            nc.sync.dma_start(out=xts, in_=xt_d[:, t * bt:(t + 1) * bt])
            xts_tiles[t] = xts

        # Critical path: first matmuls need only xts(0) + w1 pairs 0-2.
        # Split w1 across the two DMA queues; defer everything not needed
        # until after the first pair's matmuls are emitted.
        load_xts(0)
        w1sb = consts.tile([KX, 6 * 256], BF16)
        nc.sync.dma_start(out=w1sb[:, 0:768], in_=w1_d[:, 0:768])
        cpsb = consts.tile([128, 6], F32)
        nc.scalar.dma_start(out=cpsb, in_=cpack_d[:, :])
        smsb = consts.tile([1, 8 + bt], BF16)
        nc.scalar.dma_start(out=smsb, in_=small_d[:, :])
        wbsb = consts.tile([128, 1040], BF16)
        nc.scalar.dma_start(out=wbsb[:, 0:512], in_=wbig_d[:, 0:512])
        nc.sync.dma_start(out=w1sb[:, 768:1536], in_=w1_d[:, 768:1536])
        nc.scalar.dma_start(out=wbsb[:, 512:1040], in_=wbig_d[:, 512:1040])
        load_xts(1)
        load_xts(2)

        w2sb = wbsb[:, 0:512]
        wrsb = wbsb[:, 512:1024]
        whsb = wbsb[:, 1024:1040]
        bhsb = smsb[:, 0:8]
        ones_sb = smsb[:, 8:8 + bt]

        # --- per-pair state ----------------------------------------------
        ph1s, h1s, accs = {}, {}, {}
        fin_q = deque()

        def stage1(q):
            """phi1 matmuls for global pair q + psum drain (pure relu)."""
            t, p = divmod(q, 6)
            if p == 0:
                if t + 3 < nt:
                    load_xts(t + 3)
                accs[t] = sbp.tile([128, 2 * bt], BF16, tag="acc", name="acc")
            xts = xts_tiles[t]
            ph1 = psp.tile([128, 2 * bt], F32, tag="ph1", name="ph1")
            for m in range(2):
                nc.tensor.matmul(
                    ph1[:, m * bt:(m + 1) * bt],
                    w1sb[:, p * 256 + m * 128:p * 256 + (m + 1) * 128],
                    xts, start=True, stop=True,
                )
            h1 = sbp.tile([128, 2 * bt], BF16, tag="h1", name="h1", bufs=4)
            if p in (1, 3, 5):
                nc.vector.tensor_scalar_max(h1, ph1, 0.0)
            else:
                nc.scalar.activation(h1, ph1, RELU)
            ph1s[q], h1s[q] = ph1, h1

        def stage2(q):
            """phi2 m0 matmuls for pair q + ACT relu+bias consumer."""
            t, p = divmod(q, 6)
            h1, acc = h1s[q], accs[t]
            pha = psp.tile([128, bt], F32, tag="pha", name="pha")
            for k in range(2):
                nc.tensor.matmul(
                    pha, w2sb[:, (2 * k) * 128:(2 * k + 1) * 128],
                    h1[:, k * bt:(k + 1) * bt], start=(k == 0), stop=(k == 1),
                )
            if p == 0:
                nc.scalar.activation(acc[:, 0:bt], pha, RELU, bias=cpsb[:, 1:2])
            else:
                rm0 = sbp.tile([128, bt], BF16, tag="rm0", name="rm0", bufs=3)
                nc.scalar.activation(rm0, pha, RELU, bias=cpsb[:, 1:2])
                nc.gpsimd.tensor_add(acc[:, 0:bt], acc[:, 0:bt], rm0)

        def stage3(q):
            """phi2 m1 matmuls for pair q + DVE fused relu/accumulate."""
            t, p = divmod(q, 6)
            h1, acc = h1s[q], accs[t]
            phb = psp.tile([128, bt], F32, tag="phb", name="phb")
            for k in range(2):
                nc.tensor.matmul(
                    phb, w2sb[:, (2 * k + 1) * 128:(2 * k + 2) * 128],
                    h1[:, k * bt:(k + 1) * bt], start=(k == 0), stop=(k == 1),
                )
            if p == 0:
                nc.vector.tensor_scalar(
                    acc[:, bt:2 * bt], phb, cpsb[:, 0:1], 0.0,
                    op0=AMAX, op1=AADD,
                )
            else:
                nc.vector.scalar_tensor_tensor(
                    acc[:, bt:2 * bt], phb, cpsb[:, 0:1], acc[:, bt:2 * bt],
                    op0=AMAX, op1=AADD,
                )
            del ph1s[q], h1s[q]

        def finisher(t):
            """rho + heads + clip + store for tile t, as 3 weavable stages."""
            acc = accs[t]
            st = {}

            def stage_a():  # rho m0
                pr0 = psp.tile([128, bt], F32, tag="phb", name="pr0", bufs=2)
                for k in range(2):
                    nc.tensor.matmul(
                        pr0, wrsb[:, (2 * k) * 128:(2 * k + 1) * 128],
                        acc[:, k * bt:(k + 1) * bt],
                        start=(k == 0), stop=(k == 1),
                    )
                xs = sbp.tile([128, 2 * bt], BF16, tag="xs", name="xs")
                nc.scalar.activation(xs[:, 0:bt], pr0, RELU, bias=cpsb[:, 2:3])
                st["xs"] = xs

            def stage_b():  # rho m1
                pr1 = psp.tile([128, bt], F32, tag="phb", name="pr1", bufs=2)
                for k in range(2):
                    nc.tensor.matmul(
                        pr1, wrsb[:, (2 * k + 1) * 128:(2 * k + 2) * 128],
                        acc[:, k * bt:(k + 1) * bt],
                        start=(k == 0), stop=(k == 1),
                    )
                nc.scalar.activation(st["xs"][:, bt:2 * bt], pr1, RELU,
                                     bias=cpsb[:, 3:4])

            def stage_c():  # heads + clip + store
                xs = st["xs"]
                py = psp.tile([8, bt], F32, tag="pha", name="py", bufs=2)
                for k in range(2):
                    nc.tensor.matmul(py, whsb[:, k * 8:(k + 1) * 8],
                                     xs[:, k * bt:(k + 1) * bt],
                                     start=(k == 0), stop=False)
                nc.tensor.matmul(py, bhsb, ones_sb, start=False, stop=True)
                ysb = sbp.tile([8, bt], BF16, tag="ysb", name="ysb")
                nc.vector.tensor_scalar(
                    ysb, py, cpsb[0:8, 4:5], cpsb[0:8, 5:6],
                    op0=AMIN, op1=AMAX,
                )
                nc.sync.dma_start(out=y_d[:, t * bt:(t + 1) * bt], in_=ysb)

            return [stage_a, stage_b, stage_c]

        # --- master emission loop (1-pair software skew) ------------------
        for q in range(nq + 2):
            t, p = divmod(q, 6)
            if q < nq:
                stage1(q)
            if 0 <= q - 2 < nq:
                stage2(q - 2)
                stage3(q - 2)
            if p in (1, 3, 5) and fin_q:
                fin_q.popleft()()
            if q - 2 >= 0 and (q - 2) % 6 == 5:
                fin_q.extend(finisher((q - 2) // 6))
        while fin_q:
            fin_q.popleft()()

    return nc


def _get_nc(bc, bt):
    key = (bc, bt)
    if key not in _CACHE:
        nc = _build_bass(bc, bt)
        nc.finalize()
        _CACHE[key] = nc
    return _CACHE[key]


def kernel(obs, ag, g, phi_w1, phi_b1, phi_w2, phi_b2,
           rho_w1, rho_b1, mean_w, mean_b, logstd_w, logstd_b):
    obs = np.asarray(obs, np.float32)
    ag = np.asarray(ag, np.float32)
    g = np.asarray(g, np.float32)
    B = obs.shape[0]
    assert B == B_FULL, f"kernel hardcoded for B={B_FULL}, got {B}"

    packed = _pack_weights(phi_w1, phi_b1, phi_w2, phi_b2, rho_w1, rho_b1,
                           mean_w, mean_b, logstd_w, logstd_b)
    xt = _pack_xt(obs, ag, g)

    nc = _get_nc(BC, BT)
    in_maps = []
    for c in range(N_CORES):
        m = dict(packed)
        m["xt"] = np.ascontiguousarray(xt[:, c * BC:(c + 1) * BC])
        in_maps.append(m)

    import os
    trace = bool(os.environ.get("KERNEL_TRACE"))
    res = run_bass_kernel_spmd(nc, in_maps, core_ids=list(range(N_CORES)),
                               trace=trace)
    global _last_results
    _last_results = res

    y = np.concatenate(
        [np.asarray(res.results[c]["y"]) for c in range(N_CORES)], axis=1)
    out = np.ascontiguousarray(y.T.astype(np.float32))  # [B, 8]
    mean = out[:, 0:4].copy()
    logstd = out[:, 4:8].copy()
    return mean, logstd


_last_results = None
